# revision 1
# baseline (speedup 1.0000x reference)
"""BiMamba masked-LM kernel for 8 TRN2 NeuronCores.

Sharding: backbone d_inner-parallel (192 ch/core, AllReduce per layer),
lm_head row-sharded + AllGather, logits vocab-sharded (4096 rows/core).
Scan: native tensor_tensor_scan, layout (8d x 16s) partitions x L t.
Per-core 192 channels live in two tiles: half 0 = channels 0:128 (128 rows),
half 1 = channels 128:192 padded to 128 rows (rows 64:128 zeroed junk).
"""
import numpy as np

import concourse.bass as bass
import concourse.mybir as mybir
import concourse.tile as tile
from concourse.bass_utils import run_bass_kernel_spmd
from concourse.masks import make_identity

AF = mybir.ActivationFunctionType
ALU = mybir.AluOpType
F32 = mybir.dt.float32
F32R = mybir.dt.float32r
F16 = mybir.dt.float16
I32 = mybir.dt.int32

SMALL = False


class _TC(tile.TileContext):
    """TileContext whose kernel-tail drain splits its semaphore waits over
    several sync NOPs — walrus codegen rejects one instruction carrying
    them all ("Too many sync wait commands")."""

    def _drain_and_barrier(self, tick_clock, wait_clock):
        from concourse.vector_clock import ScopedClock, VectorClock
        gc = tick_clock.global_clock
        n = len(gc)
        CH = 1
        for i0 in range(0, n, CH):
            vec = [0] * n
            nz = False
            for i in range(i0, min(i0 + CH, n)):
                vec[i] = gc[i]
                nz = nz or vec[i] > 0
            if not nz:
                continue
            nop = self.nc.sync.nop(nofuse=True, hint="tail_drain_waits")
            wait_clock.add_sem_waits(nop.ins, ScopedClock({None: VectorClock(vec)}))
        self.nc.sync.drain()
        self.nc.all_engine_barrier()
        assert self.sems is not None
        popped = self.nc._tile_sem_poison_stack.pop()
        assert popped is self._sem_poison
        self.nc.clear_and_free_semaphores(list(self.sems.allocated().values()))
        self.nc.all_engine_barrier()


def dims():
    if SMALL:
        return dict(L=256, V=2048, VP=2048, D=768)
    return dict(L=2048, V=32000, VP=32768, D=768)


NC = 8
NL, DI, S, K, R = 2, 1536, 16, 4, 48
DSH = DI // NC            # 192
NJ = DSH // 8             # 24 channel-groups
DBCR = 112                # padded dbc rows: B@0:16, C@32:48, dt-rank@64:112
PASSES = [("f", 0), ("b", 0), ("f", 1), ("b", 1)]


def _nts(L):
    return [(i * 512, min(512, L - i * 512)) for i in range((L + 511) // 512)]


def f32r(ap):
    return ap


def _split_waits(nc, kmax=1):
    """Walrus codegen limits sem-wait commands per instruction; spill excess
    waits onto same-engine NoOps inserted just before the instruction."""
    for bb in nc.main_func.blocks:
        insts = bb.instructions
        out = []
        for inst in insts:
            si = inst.sync_info
            if si is not None and si.on_wait and len(si.on_wait) > 1:
                waits = list(si.on_wait)
                extra, keep = waits[:-1], waits[-1:]
                for ci, w in enumerate(extra):
                    nop = mybir.InstNoOp(name=f"{inst.name}-wsp{ci}", engine=inst.engine)
                    nop.sync_info = mybir.SyncInfo(on_wait=[w], on_update=[])
                    out.append(nop)
                si.on_wait = keep
            out.append(inst)
        insts[:] = out


def build_nc():
    d = dims()
    L, V, VP, D = d["L"], d["V"], d["VP"], d["D"]
    KT = D // 128
    MV = (VP // NC) // 128
    PSH = D // NC
    LCH = L // 128
    NTS = _nts(L)

    nc = bass.Bass()

    P = {}

    def par(nm, shape, dt=F32):
        P[nm] = nc.declare_dram_parameter(nm, shape, dt, isOutput=False)

    par("ids_f", [128, LCH], I32)
    par("ids_b", [128, LCH], I32)
    par("emb", [V, D])
    par("embT", [128, KT * (VP // NC)], F16)
    for dr, l in PASSES:
        p = f"{dr}{l}_"
        par(p + "win", [128, KT * 384], F16)
        par(p + "wout", [128, 2 * D], F16)
        par(p + "wx", [128, 2 * DBCR], F16)
        par(p + "wdt", [112, DSH], F16)
        par(p + "dtb", [128, 2])
        par(p + "cw", [128, 2 * K])
        par(p + "cb", [128, 2])
        par(p + "dpd", [128, NJ * 128], F16)
    par("lm_f", [128, KT * PSH], F16)
    par("lm_b", [128, KT * PSH], F16)
    par("pat_dA", [128, NJ * 128], F16)
    par("pat_rep", [128, NJ * 128], F16)
    par("pat_sum", [128, NJ * 128], F16)
    par("pat_B", [48, 128], F16)
    out_ext = nc.declare_dram_parameter("out", [VP // NC, L], F32, isOutput=True)

    rg = [list(range(NC))]

    with _TC(nc) as tc:
        import contextlib
        ctx = contextlib.ExitStack()
        ctx.enter_context(nc.allow_low_precision(reason="fp16 stream validated vs reference (rel err 5.6e-6)"))
        with ctx:
            pdram = ctx.enter_context(tc.tile_pool(name="pdram", bufs=1, space="DRAM"))

            def dram_t(nm, shape, shared=False):
                return pdram.tile(shape, F32, tag=nm, name=nm,
                                  addr_space=("Shared" if shared else "Local"))

            emb_fm = dram_t("emb_fm", [D, L])
            embR_fm = dram_t("embR_fm", [D, L])
            hsrc_d = {("f", 0): emb_fm, ("b", 0): embR_fm}
            bnc = {}
            for dr, l in PASSES:
                p = f"{dr}{l}_"
                bnc[p + "dbc_i"] = dram_t(p + "dbc_i", [DBCR, L])
                bnc[p + "dbc_o"] = dram_t(p + "dbc_o", [DBCR, L], shared=True)
                bnc[p + "hp_i"] = dram_t(p + "hp_i", [D, L])
                bnc[p + "hp_o"] = dram_t(p + "hp_o", [D, L], shared=True)
                hsrc_d[(dr, l + 1)] = bnc[p + "hp_o"]
            proj_i = dram_t("proj_i", [PSH, L])
            proj_o = dram_t("proj_o", [D, L], shared=True)

            pc = ctx.enter_context(tc.tile_pool(name="pc", bufs=1))
            pres = ctx.enter_context(tc.tile_pool(name="pres", bufs=2))
            pmm = ctx.enter_context(tc.tile_pool(name="pmm", bufs=2, space="PSUM"))
            pbig = ctx.enter_context(tc.tile_pool(name="pbig", bufs=1, space="PSUM"))
            ppa = ctx.enter_context(tc.tile_pool(name="ppa", bufs=1, space="PSUM"))
            ppu = ctx.enter_context(tc.tile_pool(name="ppu", bufs=1, space="PSUM"))

            ident = pc.tile([128, 128], F32, tag="ident", name="ident")
            make_identity(nc, ident)
            pat_B = pc.tile([48, 128], F16, tag="pat_B", name="pat_B")
            ones_r = pc.tile([1, 128], F16, tag="ones_r", name="ones_r")
            ones_c = pc.tile([128, 1], F16, tag="ones_c", name="ones_c")
            zeros_c = pc.tile([128, 1], F32, tag="zeros_c", name="zeros_c")
            eps_c = pc.tile([128, 1], F32, tag="eps_c", name="eps_c")
            nc.vector.memset(ones_r[:], 1.0)
            nc.vector.memset(ones_c[:], 1.0)
            nc.vector.memset(zeros_c[:], 0.0)
            nc.vector.memset(eps_c[:], 1e-5)
            nc.const_aps.aps[(F32, 0.0)] = zeros_c[:]
            nc.const_aps.aps[(F32, 1.0)] = ones_c[:]
            nc.const_aps.aps[(F32, 1e-5)] = eps_c[:]
            nc.sync.dma_start(pat_B[:], P["pat_B"][:])

            def halfpair(pool, tag, free=None, tag0=None, tag1=None, dt=F32):
                """Two (128, ...) tiles for channels 0:128 / 128:192 (+pad)."""
                fr = L if free is None else free
                return [pool.tile([128, fr], dt, tag=(tag0 or tag + "0"), name=tag + "0"),
                        pool.tile([128, fr], dt, tag=(tag1 or tag + "1"), name=tag + "1")]

            def rmsnorm_invr(hsb, ptmp):
                ssq = pbig.tile([1, L], F32, tag="big", name="ssq")
                for kt in range(KT):
                    sq = ptmp.tile([128, L], F16, tag="tmp", name="sq")
                    nc.scalar.activation(sq[:], hsb[kt][:], AF.Square)
                    for n0, nn in NTS:
                        nc.tensor.matmul(ssq[:, n0:n0 + nn], lhsT=f32r(ones_c[:]),
                                         rhs=f32r(sq[:, n0:n0 + nn]),
                                         start=(kt == 0), stop=(kt == KT - 1))
                rms = ptmp.tile([1, L], F32, tag="tmp", name="rms")
                nc.scalar.activation(rms[:], ssq[:], AF.Sqrt, scale=1.0 / D, bias=1e-5)
                inv1 = ptmp.tile([1, L], F16, tag="tmp", name="inv1")
                nc.vector.reciprocal(inv1[:], rms[:])
                pinv = pbig.tile([128, L], F32, tag="big", name="pinv")
                for n0, nn in NTS:
                    nc.tensor.matmul(pinv[:, n0:n0 + nn], lhsT=f32r(ones_r[:]),
                                     rhs=f32r(inv1[:, n0:n0 + nn]), start=True, stop=True)
                invr = ptmp.tile([128, L], F32, tag="tmp", name="invr")
                nc.scalar.activation(invr[:], pinv[:], AF.Copy)
                return invr

            # ---------------- phase 0: gather + transpose ----------------
            with tc.tile_pool(name="pg0", bufs=3) as pg0:
              for dr, dst in (("f", emb_fm), ("b", embR_fm)):
                ids_sb = pc.tile([128, LCH], I32, tag=f"ids_{dr}", name=f"ids_{dr}")
                nc.sync.dma_start(ids_sb[:], P[f"ids_{dr}"][:])
                for j in range(LCH):
                    tok = pg0.tile([128, D], F32, tag="tok", name="tok")
                    nc.gpsimd.indirect_dma_start(
                        out=tok[:], out_offset=None, in_=P["emb"][:],
                        in_offset=bass.IndirectOffsetOnAxis(ap=ids_sb[:, j:j + 1], axis=0))
                    for kt in range(KT):
                        pt = pmm.tile([128, 128], F32, tag="mm", name="pt")
                        nc.tensor.transpose(pt[:], tok[:, kt * 128:(kt + 1) * 128], ident[:])
                        st = pres.tile([128, 128], F32, tag="st", name="st")
                        nc.scalar.activation(st[:], pt[:], AF.Copy)
                        nc.sync.dma_start(
                            dst[kt * 128:(kt + 1) * 128, j * 128:(j + 1) * 128], st[:])

            # ---------------- backbone passes ----------------
            for dr, l in PASSES:
                p = f"{dr}{l}_"
                hsrc = hsrc_d[(dr, l)]
                pctx = contextlib.ExitStack()
                pwp = pctx.enter_context(tc.tile_pool(name="pwp", bufs=1))
                pstr = pctx.enter_context(tc.tile_pool(name="pstr", bufs=7))
                pwk = pctx.enter_context(tc.tile_pool(name="pwk", bufs=1))
                ptmp = pctx.enter_context(tc.tile_pool(name="ptmp", bufs=3))
                ppat = pctx.enter_context(tc.tile_pool(name="ppat", bufs=1))
                win = pwp.tile([128, KT * 384], F16, tag="wbig", name="win")
                wx = pwp.tile([128, 2 * DBCR], F16, tag="wx", name="wx")
                wdt = pwp.tile([112, DSH], F16, tag="wdt", name="wdt")
                dtb = pwp.tile([128, 2], F32, tag="dtb", name="dtb")
                cw = pwp.tile([128, 2 * K], F32, tag="cw", name="cw")
                cb = pwp.tile([128, 2], F32, tag="cb", name="cb")
                for t, nm in [(win, "win"), (wx, "wx"), (wdt, "wdt"),
                              (dtb, "dtb"), (cw, "cw"), (cb, "cb")]:
                    nc.sync.dma_start(t[:], P[p + nm][:])

                hsb = [pstr.tile([128, L], F32, tag="str", name="hsrc") for _ in range(KT)]
                for kt in range(KT):
                    nc.sync.dma_start(hsb[kt][:], hsrc[kt * 128:(kt + 1) * 128, :])
                invr = rmsnorm_invr(hsb, ptmp)
                phn = pctx.enter_context(tc.tile_pool(name="phn", bufs=KT))
                hn16 = [phn.tile([128, L], F16, tag="hn", name="hn") for _ in range(KT)]
                for kt in range(KT):
                    nc.vector.tensor_mul(hn16[kt][:], hsb[kt][:], invr[:])

                # in_proj: 3 M-tiles of 128 rows; x rows 0:192 -> x_pad pair
                # (offset K-1), z rows 192:384 -> z pair.
                xp = halfpair(pwk, "xpad", free=L + K - 1, tag0="wk0", tag1="wk1")
                z = halfpair(pwk, "z", dt=F16)
                for mt in range(3):
                    for n0, nn in NTS:
                        pz = pmm.tile([128, 512], F32, tag="mm", name="pz")
                        for kt in range(KT):
                            nc.tensor.matmul(
                                pz[:, :nn],
                                lhsT=f32r(win[:, kt * 384 + mt * 128: kt * 384 + (mt + 1) * 128]),
                                rhs=f32r(hn16[kt][:, n0:n0 + nn]),
                                start=(kt == 0), stop=(kt == KT - 1))
                        o = K - 1 + n0
                        if mt == 0:
                            nc.scalar.activation(xp[0][:, o:o + nn], pz[:, :nn], AF.Copy)
                        elif mt == 1:
                            nc.scalar.activation(xp[1][0:64, o:o + nn], pz[0:64, :nn], AF.Copy)
                            nc.vector.tensor_copy(z[0][0:64, n0:n0 + nn], pz[64:128, :nn])
                        else:
                            nc.vector.tensor_copy(z[0][64:128, n0:n0 + nn], pz[0:64, :nn])
                            nc.vector.tensor_copy(z[1][0:64, n0:n0 + nn], pz[64:128, :nn])
                for h2, rr in ((0, 128), (1, 64)):
                    nc.vector.tensor_copy(xp[h2][0:rr, 0:K - 1], xp[h2][0:rr, L:L + K - 1])

                # conv + bias + silu -> xact (pad rows of half 1 zeroed)
                xact = halfpair(pwk, "xact", dt=F16)
                nc.vector.memset(xact[1][64:128, :], 0.0)
                nc.vector.memset(z[1][64:128, :], 0.0)
                for h2, rr in ((0, 128), (1, 64)):
                    u = ptmp.tile([128, L], F32, tag="tmp", name="u")
                    u2 = ptmp.tile([128, L], F32, tag="tmp", name="u2")
                    cwv = cw[0:rr, h2 * K:(h2 + 1) * K]
                    xpv = xp[h2]
                    nc.vector.tensor_scalar(out=u[0:rr, :], in0=xpv[0:rr, 0:L],
                                            scalar1=cwv[:, 0:1], scalar2=None, op0=ALU.mult)
                    nc.vector.scalar_tensor_tensor(out=u2[0:rr, :], in0=xpv[0:rr, 1:1 + L],
                                                   scalar=cwv[:, 1:2], in1=u[0:rr, :],
                                                   op0=ALU.mult, op1=ALU.add)
                    nc.vector.scalar_tensor_tensor(out=u[0:rr, :], in0=xpv[0:rr, 2:2 + L],
                                                   scalar=cwv[:, 2:3], in1=u2[0:rr, :],
                                                   op0=ALU.mult, op1=ALU.add)
                    nc.vector.scalar_tensor_tensor(out=u2[0:rr, :], in0=xpv[0:rr, 3:3 + L],
                                                   scalar=cwv[:, 3:4], in1=u[0:rr, :],
                                                   op0=ALU.mult, op1=ALU.add)
                    nc.vector.tensor_scalar(out=u[0:rr, :], in0=u2[0:rr, :],
                                            scalar1=cb[0:rr, h2:h2 + 1], scalar2=None, op0=ALU.add)
                    sg = ptmp.tile([128, L], F32, tag="tmp", name="sg")
                    nc.scalar.activation(sg[0:rr, :], u[0:rr, :], AF.Sigmoid)
                    nc.vector.tensor_mul(xact[h2][0:rr, :], u[0:rr, :], sg[0:rr, :])

                # x_proj partial -> AllReduce -> dbc (B@0, C@32, dt@64)
                pxp = pbig.tile([DBCR, L], F32, tag="big", name="pxp")
                for n0, nn in NTS:
                    nc.tensor.matmul(pxp[:, n0:n0 + nn], lhsT=f32r(wx[:, 0:DBCR]),
                                     rhs=f32r(xact[0][:, n0:n0 + nn]), start=True, stop=False)
                    nc.tensor.matmul(pxp[:, n0:n0 + nn], lhsT=f32r(wx[0:64, DBCR:2 * DBCR]),
                                     rhs=f32r(xact[1][0:64, n0:n0 + nn]), start=False, stop=True)
                sxp = ptmp.tile([128, L], F32, tag="tmp", name="sxp")
                nc.scalar.activation(sxp[0:DBCR, :], pxp[:], AF.Copy)
                nc.sync.dma_start(bnc[p + "dbc_i"][:], sxp[0:DBCR, :])
                nc.gpsimd.collective_compute(
                    "AllReduce", ALU.add, replica_groups=rg,
                    ins=[bnc[p + "dbc_i"][:].opt()], outs=[bnc[p + "dbc_o"][:].opt()])
                dbc = pwk.tile([DBCR, L], F32, tag="dbc", name="dbc")
                nc.sync.dma_start(dbc[:], bnc[p + "dbc_o"][:])
                dbc16 = pwk.tile([DBCR, L], F16, tag="dbc16", name="dbc16")
                nc.scalar.activation(dbc16[:], dbc[:], AF.Copy)

                # delta = softplus(wdt @ dbc[64:112] + dtb); du = delta * xact
                delta = halfpair(pwk, "delta", dt=F16)
                du = halfpair(pwk, "du", dt=F16)
                nc.vector.memset(delta[1][64:128, :], 0.0)
                nc.vector.memset(du[1][64:128, :], 0.0)
                for h2, rr in ((0, 128), (1, 64)):
                    esb = ptmp.tile([128, L], F32, tag="tmp", name="esb")
                    for n0, nn in NTS:
                        pdt = pmm.tile([128, 512], F32, tag="mm", name="pdt")
                        nc.tensor.matmul(pdt[0:rr, :nn],
                                         lhsT=f32r(wdt[64:64 + R, h2 * 128:h2 * 128 + rr]),
                                         rhs=f32r(dbc16[64:64 + R, n0:n0 + nn]),
                                         start=True, stop=True)
                        nc.scalar.activation(esb[0:rr, n0:n0 + nn], pdt[0:rr, :nn],
                                             AF.Exp, bias=dtb[0:rr, h2:h2 + 1])
                    nc.scalar.activation(delta[h2][0:rr, :], esb[0:rr, :], AF.Ln, bias=1.0)
                    nc.vector.tensor_mul(du[h2][0:rr, :], delta[h2][0:rr, :], xact[h2][0:rr, :])

                # tauB / tauC replicated (row r -> s = r % 16)
                tB = pwk.tile([128, L], F16, tag="wk0", name="tB")
                tC = pwk.tile([128, L], F16, tag="wk1", name="tC")
                for tdst, off in ((tB, 0), (tC, 32)):
                    prep = pbig.tile([128, L], F32, tag="big", name="prep")
                    for n0, nn in NTS:
                        nc.tensor.matmul(prep[:, n0:n0 + nn],
                                         lhsT=f32r(pat_B[off:off + S, :]),
                                         rhs=f32r(dbc16[off:off + S, n0:n0 + nn]),
                                         start=True, stop=True)
                    nc.scalar.activation(tdst[:], prep[:], AF.Copy)

                # ---- scan stream over NJ=24 channel-groups ----
                for part in range(2):     # part 0: j 0..15 -> y1; part 1: j 16..23 -> y2
                    jlist = range(16) if part == 0 else range(16, NJ)
                    ypsum = pbig.tile([128, L], F32, tag="big", name="ypsum")
                    for j in jlist:
                        h2 = 0 if j < 16 else 1
                        jj = j if j < 16 else j - 16
                        lastj = (j == 15) if part == 0 else (j == NJ - 1)
                        jsl = slice(j * 128, (j + 1) * 128)
                        w_dA = ppat.tile([128, 128], F16, tag="pdA", name="w_dA")
                        w_rp = ppat.tile([128, 128], F16, tag="prp", name="w_rp")
                        w_sm = ppat.tile([128, 128], F16, tag="psm", name="w_sm")
                        w_dp = ppat.tile([128, 128], F16, tag="pdp", name="w_dp")
                        nc.sync.dma_start(w_dA[:], P["pat_dA"][:, jsl])
                        nc.sync.dma_start(w_rp[:], P["pat_rep"][:, jsl])
                        nc.sync.dma_start(w_sm[:], P["pat_sum"][:, jsl])
                        nc.sync.dma_start(w_dp[:], P[p + "dpd"][:, jsl])
                        dA = pstr.tile([128, L], F16, tag="str", name="dA")
                        duR = pstr.tile([128, L], F16, tag="str", name="duR")
                        dBu = pstr.tile([128, L], F16, tag="str", name="dBu")
                        hS = pstr.tile([128, L], F16, tag="str", name="hS")
                        ch = pstr.tile([128, L], F16, tag="str", name="ch")
                        for n0, nn in NTS:
                            qs = slice(n0, n0 + nn)
                            pA = ppa.tile([128, 512], F32, tag="pA", name="pA")
                            pU = ppu.tile([128, 512], F32, tag="pU", name="pU")
                            nc.tensor.matmul(pA[:, :nn], lhsT=f32r(w_dA[:]),
                                             rhs=f32r(delta[h2][:, qs]), start=True, stop=True)
                            nc.tensor.matmul(pU[:, :nn], lhsT=f32r(w_rp[:]),
                                             rhs=f32r(du[h2][:, qs]), start=True, stop=True)
                            nc.scalar.activation(dA[:, qs], pA[:, :nn], AF.Exp)
                            nc.scalar.activation(duR[:, qs], pU[:, :nn], AF.Copy)
                        # full-tile fp16 x fp16 multiply (2x DVE mode)
                        nc.vector.tensor_mul(dBu[:], duR[:], tB[:])
                        nc.vector.tensor_tensor_scan(hS[:], dA[:], dBu[:], 0.0,
                                                     ALU.mult, ALU.add)
                        nc.gpsimd.tensor_tensor(out=ch[:], in0=hS[:], in1=tC[:], op=ALU.mult)
                        for n0, nn in NTS:
                            nc.tensor.matmul(ypsum[:, n0:n0 + nn], lhsT=f32r(w_sm[:]),
                                             rhs=f32r(ch[:, n0:n0 + nn]),
                                             start=(jj == 0), stop=False)
                            nc.tensor.matmul(ypsum[:, n0:n0 + nn], lhsT=f32r(w_dp[:]),
                                             rhs=f32r(xact[h2][:, n0:n0 + nn]),
                                             start=False, stop=lastj)
                    # gate: yg = y * z * sigmoid(z), written into z tiles
                    sgz = ptmp.tile([128, L], F32, tag="tmp", name="sgz")
                    tgt = ptmp.tile([128, L], F32, tag="tmp", name="tgt")
                    nc.scalar.activation(sgz[:], z[part][:], AF.Sigmoid)
                    nc.vector.tensor_mul(tgt[:], ypsum[:], z[part][:])
                    nc.vector.tensor_mul(z[part][:], tgt[:], sgz[:])

                # out_proj + residual/8 -> AllReduce  (yg lives in z tiles)
                wout = pwp.tile([128, 2 * D], F16, tag="wbig", name="wout")
                nc.sync.dma_start(wout[:], P[p + "wout"][:])
                for n0, nn in NTS:
                    for mt in range(KT):
                        po = pmm.tile([128, 512], F32, tag="mm", name="po")
                        nc.tensor.matmul(po[:, :nn],
                                         lhsT=f32r(wout[:, mt * 128:(mt + 1) * 128]),
                                         rhs=f32r(z[0][:, n0:n0 + nn]), start=True, stop=False)
                        nc.tensor.matmul(po[:, :nn],
                                         lhsT=f32r(wout[0:64, D + mt * 128:D + (mt + 1) * 128]),
                                         rhs=f32r(z[1][0:64, n0:n0 + nn]), start=False, stop=True)
                        res = pres.tile([128, 512], F32, tag="res", name="res")
                        nc.sync.dma_start(res[:, :nn], hsrc[mt * 128:(mt + 1) * 128, n0:n0 + nn])
                        so = pres.tile([128, 512], F32, tag="so", name="so")
                        nc.vector.scalar_tensor_tensor(
                            out=so[:, :nn], in0=res[:, :nn], scalar=0.125,
                            in1=po[:, :nn], op0=ALU.mult, op1=ALU.add)
                        nc.sync.dma_start(bnc[p + "hp_i"][mt * 128:(mt + 1) * 128, n0:n0 + nn],
                                          so[:, :nn])
                nc.gpsimd.collective_compute(
                    "AllReduce", ALU.add, replica_groups=rg,
                    ins=[bnc[p + "hp_i"][:].opt()], outs=[bnc[p + "hp_o"][:].opt()])
                pctx.close()

            # ------------- final: norms, lm_head, AllGather, logits -------------
            fctx = contextlib.ExitStack()
            pfin = fctx.enter_context(tc.tile_pool(name="pfin", bufs=2 * KT))
            f1ctx = contextlib.ExitStack()
            pstr2 = f1ctx.enter_context(tc.tile_pool(name="pstr2", bufs=6))
            ptmp2 = f1ctx.enter_context(tc.tile_pool(name="ptmp2", bufs=3))
            plm = f1ctx.enter_context(tc.tile_pool(name="plm", bufs=1))
            hnf = {}
            for dr in ("f", "b"):
                hAR = hsrc_d[(dr, NL)]
                hsb = [pstr2.tile([128, L], F32, tag="str", name="hsrc") for _ in range(KT)]
                for kt in range(KT):
                    nc.sync.dma_start(hsb[kt][:], hAR[kt * 128:(kt + 1) * 128, :])
                invr = rmsnorm_invr(hsb, ptmp2)
                hnf[dr] = [pfin.tile([128, L], F16, tag="hnf", name="hnf") for _ in range(KT)]
                for kt in range(KT):
                    if dr == "f":
                        nc.vector.tensor_mul(hnf[dr][kt][:], hsb[kt][:], invr[:])
                    else:  # un-flip along t
                        nc.vector.tensor_mul(hnf[dr][kt][:], hsb[kt][:, ::-1], invr[:, ::-1])

            lmw = {}
            for dr in ("f", "b"):
                lw = plm.tile([128, KT * PSH], F16, tag=f"lm_{dr}", name=f"lm_{dr}")
                nc.sync.dma_start(lw[:], P[f"lm_{dr}"][:])
                lmw[dr] = lw
            for n0, nn in NTS:
                ppj = pmm.tile([PSH, 512], F32, tag="mm", name="ppj")
                first = True
                for dr in ("f", "b"):
                    for kt in range(KT):
                        nc.tensor.matmul(ppj[:, :nn],
                                         lhsT=f32r(lmw[dr][:, kt * PSH:(kt + 1) * PSH]),
                                         rhs=f32r(hnf[dr][kt][:, n0:n0 + nn]),
                                         start=first, stop=(dr == "b" and kt == KT - 1))
                        first = False
                spj = pres.tile([PSH, 512], F32, tag="spj", name="spj")
                nc.scalar.activation(spj[:, :nn], ppj[:, :nn], AF.Copy)
                nc.sync.dma_start(proj_i[:, n0:n0 + nn], spj[:, :nn])
            nc.gpsimd.collective_compute(
                "AllGather", ALU.bypass, replica_groups=rg,
                ins=[proj_i[:].opt()], outs=[proj_o[:].opt()])
            f1ctx.close()

            projs = [pfin.tile([128, L], F16, tag="hnf", name="projs") for _ in range(KT)]
            with tc.tile_pool(name="pcvt", bufs=2) as pcvt:
                for kt in range(KT):
                    pj32 = pcvt.tile([128, L], F32, tag="pj32", name="pj32")
                    nc.sync.dma_start(pj32[:], proj_o[kt * 128:(kt + 1) * 128, :])
                    nc.scalar.activation(projs[kt][:], pj32[:], AF.Copy)
            MVW = VP // NC
            MVH = MVW // 2
            with tc.tile_pool(name="pemb", bufs=1) as pemb:
                for vh in range(2):
                    embT = pemb.tile([128, KT * MVH], F16, tag="embT", name="embT")
                    for kt in range(KT):
                        nc.sync.dma_start(
                            embT[:, kt * MVH:(kt + 1) * MVH],
                            P["embT"][:, kt * MVW + vh * MVH: kt * MVW + (vh + 1) * MVH])
                    for mtl in range(MV // 2):
                        mt = vh * (MV // 2) + mtl
                        for n0, nn in NTS:
                            pl = pmm.tile([128, 512], F32, tag="mm", name="pl")
                            for kt in range(KT):
                                nc.tensor.matmul(
                                    pl[:, :nn],
                                    lhsT=f32r(embT[:, kt * MVH + mtl * 128: kt * MVH + (mtl + 1) * 128]),
                                    rhs=f32r(projs[kt][:, n0:n0 + nn]),
                                    start=(kt == 0), stop=(kt == KT - 1))
                            sl = pres.tile([128, 512], F32, tag="sl", name="sl")
                            if mt % 2 == 0:
                                nc.scalar.activation(sl[:, :nn], pl[:, :nn], AF.Copy)
                            else:
                                nc.vector.tensor_copy(sl[:, :nn], pl[:, :nn])
                            nc.sync.dma_start(out_ext[mt * 128:(mt + 1) * 128, n0:n0 + nn],
                                              sl[:, :nn])
            fctx.close()
    _split_waits(nc)
    return nc


# ====================== host side ======================

def _img_lhsT(w):
    """(Kdim, M) weight -> SBUF image (128, nkt*M) with K tiled by 128."""
    Kd, M = w.shape
    nkt = (Kd + 127) // 128
    img = np.zeros((128, nkt * M), np.float32)
    for kt in range(nkt):
        rows = min(128, Kd - kt * 128)
        img[:rows, kt * M:(kt + 1) * M] = w[kt * 128:kt * 128 + rows]
    return img


def _img_cols2(v):
    img = np.zeros((128, 2), np.float32)
    img[:, 0] = v[0:128]
    img[0:64, 1] = v[128:192]
    return img


def _prep_core(inputs, k, d):
    L, V, VP, D = d["L"], d["V"], d["VP"], d["D"]
    KT = D // 128
    PSH = D // NC
    LCH = L // 128
    ids = np.asarray(inputs["input_ids"]).reshape(L).astype(np.int32)
    emb = np.asarray(inputs["embedding"], np.float32)
    m = {}
    m["ids_f"] = np.ascontiguousarray(ids.reshape(LCH, 128).T)
    m["ids_b"] = np.ascontiguousarray(ids[::-1].reshape(LCH, 128).T)
    m["emb"] = emb
    embP = np.zeros((VP, D), np.float32)
    embP[:V] = emb
    m["embT"] = _img_lhsT(np.ascontiguousarray(embP[k * (VP // NC):(k + 1) * (VP // NC)].T))

    c0, c1 = k * DSH, (k + 1) * DSH
    for dr in ("f", "b"):
        for l in range(NL):
            p = f"{dr}{l}_"
            g = lambda nm: np.asarray(inputs[f"{dr}_{nm}"][l], np.float32)
            W = np.concatenate([g("in_proj")[c0:c1], g("in_proj")[DI + c0:DI + c1]], 0)
            W = W * np.asarray(inputs[f"{dr}_norm_w"][l], np.float32)[None, :]
            m[p + "win"] = _img_lhsT(np.ascontiguousarray(W.T))
            m[p + "wout"] = _img_lhsT(np.ascontiguousarray(g("out_proj")[:, c0:c1].T))
            xpT = np.ascontiguousarray(g("x_proj")[:, c0:c1].T)   # (192, 80)
            xpP = np.zeros((DSH, DBCR), np.float32)
            xpP[:, 0:S] = xpT[:, R:R + S]
            xpP[:, 32:32 + S] = xpT[:, R + S:R + 2 * S]
            xpP[:, 64:64 + R] = xpT[:, 0:R]
            m[p + "wx"] = _img_lhsT(xpP)
            wdtP = np.zeros((112, DSH), np.float32)
            wdtP[64:64 + R] = g("dt_w")[c0:c1].T
            m[p + "wdt"] = wdtP
            m[p + "dtb"] = _img_cols2(g("dt_b")[c0:c1])
            cwk = g("conv_w")[c0:c1]
            m[p + "cw"] = np.zeros((128, 2 * K), np.float32)
            m[p + "cw"][:, 0:K] = cwk[0:128]
            m[p + "cw"][0:64, K:2 * K] = cwk[128:192]
            m[p + "cb"] = _img_cols2(g("conv_b")[c0:c1])
            dp = g("Dp")[c0:c1]
            dpd = np.zeros((128, NJ * 128), np.float32)
            for j in range(NJ):
                for q in range(8):
                    ch_ = (8 * j + q) % 128   # row within the half tile
                    dpd[ch_, j * 128 + ch_] = dp[8 * j + q]
            m[p + "dpd"] = dpd
    lm = np.asarray(inputs["lm_head_proj"], np.float32)
    nf_f = np.asarray(inputs["f_norm_f"], np.float32)
    nf_b = np.asarray(inputs["b_norm_f"], np.float32)
    r0, r1 = k * PSH, (k + 1) * PSH
    m["lm_f"] = _img_lhsT(np.ascontiguousarray((lm[r0:r1, :D] * nf_f[None, :]).T))
    m["lm_b"] = _img_lhsT(np.ascontiguousarray((lm[r0:r1, D:] * nf_b[None, :]).T))

    # patterns: scan-tile row m -> (dloc = m//16, s = m%16); channel-group j
    pat_dA = np.zeros((128, NJ * 128), np.float32)
    pat_rep = np.zeros((128, NJ * 128), np.float32)
    pat_sum = np.zeros((128, NJ * 128), np.float32)
    pat_B = np.zeros((48, 128), np.float32)
    for mm_ in range(128):
        dloc, s = mm_ // 16, mm_ % 16
        pat_B[s, mm_] = 1.0
        pat_B[32 + s, mm_] = 1.0
        for j in range(NJ):
            krow = (8 * j + dloc) % 128     # row of delta/du half tile
            pat_dA[krow, j * 128 + mm_] = -(s + 1)
            pat_rep[krow, j * 128 + mm_] = 1.0
    for r in range(128):
        dloc = r // 16
        for j in range(NJ):
            mrow = (8 * j + dloc) % 128     # row of ypsum
            pat_sum[r, j * 128 + mrow] = 1.0
    m["pat_dA"], m["pat_rep"], m["pat_sum"], m["pat_B"] = pat_dA, pat_rep, pat_sum, pat_B
    f16keys = ["embT", "lm_f", "lm_b", "pat_dA", "pat_rep", "pat_sum", "pat_B"]
    for dr in ("f", "b"):
        for l in range(NL):
            pp_ = f"{dr}{l}_"
            f16keys += [pp_ + "win", pp_ + "wout", pp_ + "wx", pp_ + "wdt", pp_ + "dpd"]
    for k_ in f16keys:
        m[k_] = m[k_].astype(np.float16)
    return m


_NC_CACHE = {}
TRACE = False
LAST_EXEC_NS = None
LAST_RESULTS = None


def kernel(**inputs):
    global LAST_EXEC_NS, LAST_RESULTS
    d = dims()
    key = "small" if SMALL else "full"
    if key not in _NC_CACHE:
        _NC_CACHE[key] = build_nc()
    ncg = _NC_CACHE[key]
    in_maps = [_prep_core(inputs, k, d) for k in range(NC)]
    res = run_bass_kernel_spmd(ncg, in_maps, core_ids=list(range(NC)), trace=TRACE)
    LAST_EXEC_NS = res.exec_time_ns
    LAST_RESULTS = res
    L, V, VP = d["L"], d["V"], d["VP"]
    full = np.concatenate([res.results[k]["out"] for k in range(NC)], axis=0)  # (VP, L)
    return np.ascontiguousarray(full[:V].T[None])


def timed_run(inputs, iters=4):
    """Measure per-call wall time of the compiled SPMD executable with
    pre-staged device inputs (no donation, no re-transfer). Returns
    (best_seconds, results_list)."""
    import time
    import jax
    from jax.sharding import Mesh, PartitionSpec
    from jax.experimental.shard_map import shard_map
    from concourse import bass2jax, mybir as mb

    d = dims()
    key = "small" if SMALL else "full"
    if key not in _NC_CACHE:
        _NC_CACHE[key] = build_nc()
    ncg = _NC_CACHE[key]
    in_maps = [_prep_core(inputs, k, d) for k in range(NC)]
    bass2jax.install_neuronx_cc_hook()
    partition_name = ncg.partition_id_tensor.name if ncg.partition_id_tensor else None
    in_names, out_names, out_avals, zero_outs = [], [], [], []
    for alloc in ncg.m.functions[0].allocations:
        if not isinstance(alloc, mb.MemoryLocationSet):
            continue
        name = alloc.memorylocations[0].name
        if alloc.kind == "ExternalInput":
            if name != partition_name:
                in_names.append(name)
        elif alloc.kind == "ExternalOutput":
            shape = tuple(alloc.tensor_shape)
            dtype = mb.dt.np(alloc.dtype)
            out_names.append(name)
            out_avals.append(jax.core.ShapedArray(shape, dtype))
            zero_outs.append(np.zeros(shape, dtype))
    n_params = len(in_names)
    all_names = in_names + out_names
    if partition_name is not None:
        all_names = all_names + [partition_name]

    def _body(*args):
        operands = list(args)
        if partition_name is not None:
            operands.append(bass2jax.partition_id_tensor())
        outs = bass2jax._bass_exec_p.bind(
            *operands, out_avals=tuple(out_avals), in_names=tuple(all_names),
            out_names=tuple(out_names), lowering_input_output_aliases=(),
            sim_require_finite=True, sim_require_nnan=True, nc=ncg)
        return tuple(outs)

    devices = jax.devices()[:NC]
    mesh = Mesh(np.asarray(devices), ("core",))
    nin = n_params + len(zero_outs)
    sharded = jax.jit(shard_map(_body, mesh=mesh,
                                in_specs=(PartitionSpec("core"),) * nin,
                                out_specs=(PartitionSpec("core"),) * len(out_names),
                                check_rep=False), keep_unused=True)
    per_core = [[np.asarray(m[nm]) for nm in in_names] for m in in_maps]
    concat_in = [np.concatenate([per_core[c][i] for c in range(NC)], axis=0)
                 for i in range(n_params)]
    concat_zeros = [np.zeros((NC * z.shape[0], *z.shape[1:]), z.dtype)
                    for z in zero_outs]
    shardings = [jax.sharding.NamedSharding(mesh, PartitionSpec("core"))] * nin
    staged = [jax.device_put(a, s) for a, s in zip(concat_in + concat_zeros, shardings)]
    out = sharded(*staged)
    jax.block_until_ready(out)
    best = float("inf")
    for _ in range(iters):
        t0 = time.perf_counter()
        out = sharded(*staged)
        jax.block_until_ready(out)
        best = min(best, time.perf_counter() - t0)
    res = [{nm: np.asarray(out[i]).reshape(NC, *out_avals[i].shape)[c]
            for i, nm in enumerate(out_names)} for c in range(NC)]
    return best, res



# revision 10
# speedup vs baseline: 2.2435x; 2.2435x over previous
"""BiMamba masked-LM kernel for 8 TRN2 NeuronCores — v2.

Sharding: backbone d_inner-parallel (192 ch/core), fp16 collectives:
dbc AllReduce (80xL f16), residual stream ReduceScatter+AllGather (f16).
lm_head replicated per-core after the last AllGather (no final collective),
logits vocab-sharded (4096 rows/core).

Issue order interleaves the two independent direction chains per layer
(f-pre, b-pre, f-scan, f-post, b-scan, b-post) so each direction's
collectives hide under the other direction's compute.

Scan: native tensor_tensor_scan (fp16 4x DVE mode), layout (8d x 16s)
partitions x L t. Per-core 192 channels in two half tiles (128 + 64pad).
"""
import numpy as np

import concourse.bass as bass
import concourse.mybir as mybir
import concourse.tile as tile
from concourse.bass_utils import run_bass_kernel_spmd
from concourse.masks import make_identity

AF = mybir.ActivationFunctionType
ALU = mybir.AluOpType
F32 = mybir.dt.float32
F16 = mybir.dt.float16
I32 = mybir.dt.int32

SMALL = False


class _TC(tile.TileContext):
    """TileContext whose kernel-tail drain splits its semaphore waits over
    several sync NOPs — walrus codegen rejects one instruction carrying
    them all ("Too many sync wait commands")."""

    def _drain_and_barrier(self, tick_clock, wait_clock):
        from concourse.vector_clock import ScopedClock, VectorClock
        gc = tick_clock.global_clock
        n = len(gc)
        CH = 1
        for i0 in range(0, n, CH):
            vec = [0] * n
            nz = False
            for i in range(i0, min(i0 + CH, n)):
                vec[i] = gc[i]
                nz = nz or vec[i] > 0
            if not nz:
                continue
            nop = self.nc.sync.nop(nofuse=True, hint="tail_drain_waits")
            wait_clock.add_sem_waits(nop.ins, ScopedClock({None: VectorClock(vec)}))
        self.nc.sync.drain()
        self.nc.all_engine_barrier()
        assert self.sems is not None
        popped = self.nc._tile_sem_poison_stack.pop()
        assert popped is self._sem_poison
        self.nc.clear_and_free_semaphores(list(self.sems.allocated().values()))
        self.nc.all_engine_barrier()


def dims():
    if SMALL:
        return dict(L=256, V=2048, VP=2048, D=768)
    return dict(L=2048, V=32000, VP=32768, D=768)


NC = 8
NL, DI, S, K, R = 2, 1536, 16, 4, 48
DSH = DI // NC            # 192
NJ = DSH // 8             # 24 channel-groups
DBCR = 112                # padded dbc rows: B@0:16, C@32:48, dt-rank@64:112
PASSES = [("f", 0), ("b", 0), ("f", 1), ("b", 1)]


def _nts(L):
    return [(i * 512, min(512, L - i * 512)) for i in range((L + 511) // 512)]


def _split_waits(nc, kmax=1):
    """Walrus codegen limits sem-wait commands per instruction; spill excess
    waits onto same-engine NoOps inserted just before the instruction."""
    for bb in nc.main_func.blocks:
        insts = bb.instructions
        out = []
        for inst in insts:
            si = inst.sync_info
            if si is not None and si.on_wait and len(si.on_wait) > 1:
                waits = list(si.on_wait)
                extra, keep = waits[:-1], waits[-1:]
                for ci, w in enumerate(extra):
                    nop = mybir.InstNoOp(name=f"{inst.name}-wsp{ci}", engine=inst.engine)
                    nop.sync_info = mybir.SyncInfo(on_wait=[w], on_update=[])
                    out.append(nop)
                si.on_wait = keep
            out.append(inst)
        insts[:] = out


def build_nc():
    d = dims()
    L, V, VP, D = d["L"], d["V"], d["VP"], d["D"]
    KT = D // 128             # 6
    MV = (VP // NC) // 128    # 32 vocab M-tiles
    LCH = L // 128
    NTS = _nts(L)

    nc = bass.Bass()

    P = {}

    def par(nm, shape, dt=F32):
        P[nm] = nc.declare_dram_parameter(nm, shape, dt, isOutput=False)

    par("ids", [128, LCH], I32)
    par("emb", [V, D])
    par("embT", [128, KT * (VP // NC)], F16)
    for dr, l in PASSES:
        p = f"{dr}{l}_"
        par(p + "win", [128, KT * 384], F16)
        par(p + "wout", [128, 2 * D], F16)
        par(p + "wx", [128, 2 * DBCR], F16)
        par(p + "wdt", [DBCR, DSH], F16)
        par(p + "dtb", [128, 2])
        par(p + "cw", [128, 2 * K])
        par(p + "cb", [128, 2])
        par(p + "dpc", [128, 2])
    par("lm_f", [128, KT * D], F16)
    par("lm_b", [128, KT * D], F16)
    par("pat_dA", [128, NJ * 128], F16)
    par("pat_rep", [128, NJ * 128], F16)
    par("pat_sum", [128, NJ * 128], F16)
    par("pat_B", [48, 128], F16)
    out_ext = nc.declare_dram_parameter("out", [VP // NC, L], F32, isOutput=True)

    rg = [list(range(NC))]

    with _TC(nc) as tc:
        import contextlib
        ctx = contextlib.ExitStack()
        ctx.enter_context(nc.allow_low_precision(reason="fp16 stream validated vs reference"))
        with ctx:
            pdram = ctx.enter_context(tc.tile_pool(name="pdram", bufs=1, space="DRAM"))

            def dram_t(nm, shape, dt=F16, shared=False):
                return pdram.tile(shape, dt, tag=nm, name=nm,
                                  addr_space=("Shared" if shared else "Local"))

            bnc = {}
            for dr, l in PASSES:
                p = f"{dr}{l}_"
                bnc[p + "dbc_i"] = dram_t(p + "dbc_i", [DBCR, L])
                bnc[p + "dbc_o"] = dram_t(p + "dbc_o", [DBCR, L], shared=True)
                bnc[p + "hp_i"] = dram_t(p + "hp_i", [D, L])
                bnc[p + "hp_rs"] = dram_t(p + "hp_rs", [D // NC, L])
                bnc[p + "hp_o"] = dram_t(p + "hp_o", [D, L], shared=True)
            emb_fm = {dr: dram_t(f"emb_fm_{dr}", [D, L]) for dr in ("f", "b")}

            # ---------------- constant / persistent pools ----------------
            pc = ctx.enter_context(tc.tile_pool(name="pc", bufs=1))
            # PSUM pools: pmm(2) + pbig(4) + ppa(2) = 8 banks total
            pmm = ctx.enter_context(tc.tile_pool(name="pmm", bufs=2, space="PSUM"))
            pbig = ctx.enter_context(tc.tile_pool(name="pbig", bufs=1, space="PSUM"))
            ppa = ctx.enter_context(tc.tile_pool(name="ppa", bufs=2, space="PSUM"))
            pres = ctx.enter_context(tc.tile_pool(name="pres", bufs=3))
            # scan patterns on the right side so emb pool below can close early
            rctx = contextlib.ExitStack()
            ppat = rctx.enter_context(tc.tile_pool(name="ppat", bufs=1, side="right"))

            ident = pc.tile([128, 128], F32, tag="ident", name="ident")
            make_identity(nc, ident)
            pat_B = pc.tile([48, 128], F16, tag="pat_B", name="pat_B")
            ones_r = pc.tile([1, 128], F16, tag="ones_r", name="ones_r")
            ones_c = pc.tile([128, 1], F16, tag="ones_c", name="ones_c")
            zeros_c = pc.tile([128, 1], F32, tag="zeros_c", name="zeros_c")
            eps_c = pc.tile([128, 1], F32, tag="eps_c", name="eps_c")
            nc.vector.memset(ones_r[:], 1.0)
            nc.vector.memset(ones_c[:], 1.0)
            nc.vector.memset(zeros_c[:], 0.0)
            nc.vector.memset(eps_c[:], 1e-5)
            nc.const_aps.aps[(F32, 0.0)] = zeros_c[:]
            nc.const_aps.aps[(F32, 1.0)] = ones_c[:]
            nc.const_aps.aps[(F32, 1e-5)] = eps_c[:]
            nc.sync.dma_start(pat_B[:], P["pat_B"][:])

            patdA = ppat.tile([128, NJ * 128], F16, tag="patdA", name="patdA")
            patrp = ppat.tile([128, NJ * 128], F16, tag="patrp", name="patrp")
            patsm = ppat.tile([128, NJ * 128], F16, tag="patsm", name="patsm")
            nc.sync.dma_start(patdA[:], P["pat_dA"][:])
            nc.sync.dma_start(patrp[:], P["pat_rep"][:])
            nc.sync.dma_start(patsm[:], P["pat_sum"][:])

            def halfpair(pool, tag, free=None, dt=F16):
                fr = L if free is None else free
                return [pool.tile([128, fr], dt, tag=tag + "0", name=tag + "0"),
                        pool.tile([128, fr], dt, tag=tag + "1", name=tag + "1")]

            def rmsnorm_invr(hsb, ptmp, ptmp1):
                """1/rms as a (128,L) fp16 tile (norm weight folded in win/lm)."""
                ssq = pbig.tile([1, L], F32, tag="big", name="ssq")
                for kt in range(KT):
                    sq = ptmp.tile([128, L], F16, tag="tmp", name="sq")
                    nc.scalar.activation(sq[:], hsb[kt][:], AF.Square)
                    for n0, nn in NTS:
                        nc.tensor.matmul(ssq[:, n0:n0 + nn], lhsT=ones_c[:],
                                         rhs=sq[:, n0:n0 + nn],
                                         start=(kt == 0), stop=(kt == KT - 1))
                rms = ptmp1.tile([1, L], F16, tag="tmp1", name="rms")
                nc.scalar.activation(rms[:], ssq[:], AF.Sqrt, scale=1.0 / D, bias=1e-5)
                inv1 = ptmp1.tile([1, L], F16, tag="tmp1b", name="inv1")
                nc.vector.reciprocal(inv1[:], rms[:])
                pinv = pbig.tile([128, L], F32, tag="big", name="pinv")
                for n0, nn in NTS:
                    nc.tensor.matmul(pinv[:, n0:n0 + nn], lhsT=ones_r[:],
                                     rhs=inv1[:, n0:n0 + nn], start=True, stop=True)
                invr = ptmp.tile([128, L], F16, tag="tmp", name="invr")
                nc.scalar.activation(invr[:], pinv[:], AF.Copy)
                return invr

            # ---------------- phase 0: single gather + transpose ----------------
            pembp_ctx = contextlib.ExitStack()
            pembp = pembp_ctx.enter_context(tc.tile_pool(name="pembp", bufs=1, side="right"))
            embf = [pembp.tile([128, L], F16, tag=f"embf{kt}", name="embf") for kt in range(KT)]
            embb = [pembp.tile([128, L], F16, tag=f"embb{kt}", name="embb") for kt in range(KT)]
            with tc.tile_pool(name="pg0", bufs=3) as pg0:
                ids_sb = pc.tile([128, LCH], I32, tag="ids", name="ids")
                nc.sync.dma_start(ids_sb[:], P["ids"][:])
                for j in range(LCH):
                    tok = pg0.tile([128, D], F32, tag="tok", name="tok")
                    nc.gpsimd.indirect_dma_start(
                        out=tok[:], out_offset=None, in_=P["emb"][:],
                        in_offset=bass.IndirectOffsetOnAxis(ap=ids_sb[:, j:j + 1], axis=0))
                    for kt in range(KT):
                        pt = pmm.tile([128, 128], F32, tag="mm", name="pt")
                        nc.tensor.transpose(pt[:], tok[:, kt * 128:(kt + 1) * 128], ident[:])
                        nc.scalar.activation(embf[kt][:, j * 128:(j + 1) * 128], pt[:], AF.Copy)
            for kt in range(KT):
                nc.vector.tensor_copy(embb[kt][:], embf[kt][:, ::-1])
            # DRAM copies for the residual path (SBUF tiles close after b-pre)
            for dr, tl in (("f", embf), ("b", embb)):
                for kt in range(KT):
                    nc.sync.dma_start(emb_fm[dr][kt * 128:(kt + 1) * 128, :], tl[kt][:])
            embd = {"f": embf, "b": embb}

            # ---------------- backbone: 2 layers, f/b interleaved ----------------
            hp_o = {}
            res_src = {("f", 0): emb_fm["f"], ("b", 0): emb_fm["b"]}
            for l in range(NL):
                lctx = contextlib.ExitStack()
                st = {}
                for dr in ("f", "b"):
                    p = f"{dr}{l}_"
                    pwp = lctx.enter_context(tc.tile_pool(name=f"pwp{dr}{l}", bufs=1))
                    pwk = lctx.enter_context(tc.tile_pool(name=f"pwk{dr}{l}", bufs=1))
                    wx = pwp.tile([128, 2 * DBCR], F16, tag="wx", name="wx")
                    wdt = pwp.tile([DBCR, DSH], F16, tag="wdt", name="wdt")
                    dtb = pwp.tile([128, 2], F32, tag="dtb", name="dtb")
                    cw = pwp.tile([128, 2 * K], F32, tag="cw", name="cw")
                    cb = pwp.tile([128, 2], F32, tag="cb", name="cb")
                    dpc = pwp.tile([128, 2], F32, tag="dpc", name="dpc")
                    wout = pwp.tile([128, 2 * D], F16, tag="wout", name="wout")
                    for t, nm in [(wx, "wx"), (wdt, "wdt"), (dtb, "dtb"),
                                  (cw, "cw"), (cb, "cb"), (dpc, "dpc"), (wout, "wout")]:
                        nc.sync.dma_start(t[:], P[p + nm][:])
                    st[dr] = dict(pwp=pwp, pwk=pwk, wdt=wdt, dtb=dtb, dpc=dpc, wout=wout, wx=wx,
                                  cw=cw, cb=cb)

                # ---- PRE stages ----
                for dr in ("f", "b"):
                    p = f"{dr}{l}_"
                    sd = st[dr]
                    prectx = contextlib.ExitStack()
                    ptmp = prectx.enter_context(tc.tile_pool(name=f"ptmp{dr}{l}", bufs=2))
                    ptmp1 = prectx.enter_context(tc.tile_pool(name=f"ptmp1{dr}{l}", bufs=1))
                    win = ptmp1.tile([128, KT * 384], F16, tag="win", name="win")
                    nc.sync.dma_start(win[:], P[p + "win"][:])
                    if l == 0:
                        hsb = embd[dr]
                    else:
                        phsb = prectx.enter_context(
                            tc.tile_pool(name=f"phsb{dr}{l}", bufs=KT))
                        hsb = [phsb.tile([128, L], F16, tag="hs", name="hs") for _ in range(KT)]
                        src = hp_o[(dr, l - 1)]
                        for kt in range(KT):
                            nc.sync.dma_start(hsb[kt][:], src[kt * 128:(kt + 1) * 128, :])
                    invr = rmsnorm_invr(hsb, ptmp, ptmp1)
                    phn = prectx.enter_context(tc.tile_pool(name=f"phn{dr}{l}", bufs=KT))
                    hn16 = [phn.tile([128, L], F16, tag="hn", name="hn") for _ in range(KT)]
                    for kt in range(KT):
                        nc.vector.tensor_mul(hn16[kt][:], hsb[kt][:], invr[:])

                    # in_proj: 3 M-tiles of 128 rows; x rows 0:192 -> xp pair
                    # (offset K-1), z rows 192:384 -> z pair.
                    xp = halfpair(ptmp1, "xpad", free=L + K - 1)
                    z = halfpair(sd["pwk"], "z")
                    for mt in range(3):
                        for n0, nn in NTS:
                            pz = pmm.tile([128, 512], F32, tag="mm", name="pz")
                            for kt in range(KT):
                                nc.tensor.matmul(
                                    pz[:, :nn],
                                    lhsT=win[:, kt * 384 + mt * 128: kt * 384 + (mt + 1) * 128],
                                    rhs=hn16[kt][:, n0:n0 + nn],
                                    start=(kt == 0), stop=(kt == KT - 1))
                            o = K - 1 + n0
                            if mt == 0:
                                nc.scalar.activation(xp[0][:, o:o + nn], pz[:, :nn], AF.Copy)
                            elif mt == 1:
                                nc.scalar.activation(xp[1][0:64, o:o + nn], pz[0:64, :nn], AF.Copy)
                                nc.vector.tensor_copy(z[0][0:64, n0:n0 + nn], pz[64:128, :nn])
                            else:
                                nc.vector.tensor_copy(z[0][64:128, n0:n0 + nn], pz[0:64, :nn])
                                nc.vector.tensor_copy(z[1][0:64, n0:n0 + nn], pz[64:128, :nn])
                    for h2, rr in ((0, 128), (1, 64)):
                        nc.vector.tensor_copy(xp[h2][0:rr, 0:K - 1], xp[h2][0:rr, L:L + K - 1])

                    # conv + bias + silu -> xact (pad rows of half 1 zeroed)
                    xact = halfpair(sd["pwk"], "xact")
                    nc.vector.memset(xact[1][64:128, :], 0.0)
                    nc.vector.memset(z[1][64:128, :], 0.0)
                    for h2, rr in ((0, 128), (1, 64)):
                        u = ptmp1.tile([128, L], F16, tag="cu", name="u")
                        u2 = ptmp1.tile([128, L], F16, tag="cu2", name="u2")
                        cwv = sd["cw"][0:rr, h2 * K:(h2 + 1) * K]
                        xpv = xp[h2]
                        nc.vector.tensor_scalar(out=u[0:rr, :], in0=xpv[0:rr, 0:L],
                                                scalar1=cwv[:, 0:1], scalar2=None, op0=ALU.mult)
                        nc.vector.scalar_tensor_tensor(out=u2[0:rr, :], in0=xpv[0:rr, 1:1 + L],
                                                       scalar=cwv[:, 1:2], in1=u[0:rr, :],
                                                       op0=ALU.mult, op1=ALU.add)
                        nc.vector.scalar_tensor_tensor(out=u[0:rr, :], in0=xpv[0:rr, 2:2 + L],
                                                       scalar=cwv[:, 2:3], in1=u2[0:rr, :],
                                                       op0=ALU.mult, op1=ALU.add)
                        nc.vector.scalar_tensor_tensor(out=u2[0:rr, :], in0=xpv[0:rr, 3:3 + L],
                                                       scalar=cwv[:, 3:4], in1=u[0:rr, :],
                                                       op0=ALU.mult, op1=ALU.add)
                        nc.vector.tensor_scalar(out=u[0:rr, :], in0=u2[0:rr, :],
                                                scalar1=sd["cb"][0:rr, h2:h2 + 1], scalar2=None,
                                                op0=ALU.add)
                        nc.scalar.activation(xact[h2][0:rr, :], u[0:rr, :], AF.Silu)

                    # x_proj partial -> fp16 AllReduce
                    pxp = pbig.tile([DBCR, L], F32, tag="big", name="pxp")
                    for n0, nn in NTS:
                        nc.tensor.matmul(pxp[:, n0:n0 + nn], lhsT=sd["wx"][:, 0:DBCR],
                                         rhs=xact[0][:, n0:n0 + nn], start=True, stop=False)
                        nc.tensor.matmul(pxp[:, n0:n0 + nn], lhsT=sd["wx"][0:64, DBCR:2 * DBCR],
                                         rhs=xact[1][0:64, n0:n0 + nn], start=False, stop=True)
                    sxp = ptmp.tile([128, L], F16, tag="tmp", name="sxp")
                    nc.scalar.activation(sxp[0:DBCR, :], pxp[:], AF.Copy)
                    nc.sync.dma_start(bnc[p + "dbc_i"][:], sxp[0:DBCR, :])
                    nc.gpsimd.collective_compute(
                        "AllReduce", ALU.add, replica_groups=rg,
                        ins=[bnc[p + "dbc_i"][:].opt()], outs=[bnc[p + "dbc_o"][:].opt()])
                    prectx.close()
                    if l == 0 and dr == "b":
                        pembp_ctx.close()
                    sd["xact"] = xact
                    sd["z"] = z

                # ---- SCAN + POST stages ----
                for dr in ("f", "b"):
                    p = f"{dr}{l}_"
                    sd = st[dr]
                    xact, z = sd["xact"], sd["z"]
                    sctx = contextlib.ExitStack()
                    ptmp = sctx.enter_context(tc.tile_pool(name=f"stmp{dr}{l}", bufs=1))
                    pdl = sctx.enter_context(tc.tile_pool(name=f"pdl{dr}{l}", bufs=1))
                    pstr = sctx.enter_context(tc.tile_pool(name=f"pstr{dr}{l}", bufs=8))

                    dbc16 = pdl.tile([DBCR, L], F16, tag="dbc16", name="dbc16")
                    nc.sync.dma_start(dbc16[:], bnc[p + "dbc_o"][:])

                    # delta = softplus(wdt @ dbc[64:112] + dtb); du = delta * xact
                    delta = halfpair(pdl, "delta")
                    du = halfpair(pdl, "du")
                    nc.vector.memset(delta[1][64:128, :], 0.0)
                    nc.vector.memset(du[1][64:128, :], 0.0)
                    for h2, rr in ((0, 128), (1, 64)):
                        esb = ptmp.tile([128, L], F32, tag="esb", name="esb")
                        for n0, nn in NTS:
                            pdt = pmm.tile([128, 512], F32, tag="mm", name="pdt")
                            nc.tensor.matmul(pdt[0:rr, :nn],
                                             lhsT=sd["wdt"][64:64 + R, h2 * 128:h2 * 128 + rr],
                                             rhs=dbc16[64:64 + R, n0:n0 + nn],
                                             start=True, stop=True)
                            nc.scalar.activation(esb[0:rr, n0:n0 + nn], pdt[0:rr, :nn],
                                                 AF.Exp, bias=sd["dtb"][0:rr, h2:h2 + 1])
                        nc.scalar.activation(delta[h2][0:rr, :], esb[0:rr, :], AF.Ln, bias=1.0)
                        nc.vector.tensor_mul(du[h2][0:rr, :], delta[h2][0:rr, :], xact[h2][0:rr, :])

                    # tauB / tauC replicated (row r -> s = r % 16)
                    tB = pdl.tile([128, L], F16, tag="tB", name="tB")
                    tC = pdl.tile([128, L], F16, tag="tC", name="tC")
                    for tdst, off in ((tB, 0), (tC, 32)):
                        for n0, nn in NTS:
                            prep = pmm.tile([128, 512], F32, tag="mm", name="prep")
                            nc.tensor.matmul(prep[:, :nn],
                                             lhsT=pat_B[off:off + S, :],
                                             rhs=dbc16[off:off + S, n0:n0 + nn],
                                             start=True, stop=True)
                            nc.scalar.activation(tdst[:, n0:n0 + nn], prep[:, :nn], AF.Copy)

                    # ---- scan stream over NJ=24 channel-groups ----
                    for part in range(2):   # part 0: j 0..15 -> z[0]; part 1: j 16..23 -> z[1]
                        jlist = range(16) if part == 0 else range(16, NJ)
                        ypsum = pbig.tile([128, L], F32, tag="big", name="ypsum")
                        for j in jlist:
                            h2 = 0 if j < 16 else 1
                            firstj = (j == 0) if part == 0 else (j == 16)
                            lastj = (j == 15) if part == 0 else (j == NJ - 1)
                            jsl = slice(j * 128, (j + 1) * 128)
                            dA = pstr.tile([128, L], F16, tag="str", name="dA")
                            dBu = pstr.tile([128, L], F16, tag="str", name="dBu")
                            hS = pstr.tile([128, L], F16, tag="str", name="hS")
                            ch = pstr.tile([128, L], F16, tag="str", name="ch")
                            for n0, nn in NTS:
                                qs = slice(n0, n0 + nn)
                                pA = ppa.tile([128, 512], F32, tag="pA", name="pA")
                                pU = pmm.tile([128, 512], F32, tag="mm", name="pU")
                                nc.tensor.matmul(pA[:, :nn], lhsT=patdA[:, jsl],
                                                 rhs=delta[h2][:, qs], start=True, stop=True)
                                nc.tensor.matmul(pU[:, :nn], lhsT=patrp[:, jsl],
                                                 rhs=du[h2][:, qs], start=True, stop=True)
                                nc.scalar.activation(dA[:, qs], pA[:, :nn], AF.Exp)
                                nc.vector.tensor_mul(dBu[:, qs], pU[:, :nn], tB[:, qs])
                            nc.vector.tensor_tensor_scan(hS[:], dA[:], dBu[:], 0.0,
                                                         ALU.mult, ALU.add)
                            nc.vector.tensor_mul(ch[:], hS[:], tC[:])
                            for n0, nn in NTS:
                                nc.tensor.matmul(ypsum[:, n0:n0 + nn], lhsT=patsm[:, jsl],
                                                 rhs=ch[:, n0:n0 + nn],
                                                 start=firstj, stop=lastj)
                        # gate: yg = (ypsum + Dp*xact) * silu(z) -> z tile
                        sgz = ptmp.tile([128, L], F16, tag="sgz", name="sgz")
                        t1 = ptmp.tile([128, L], F16, tag="t1", name="t1")
                        nc.scalar.activation(sgz[:], z[part][:], AF.Silu)
                        nc.vector.scalar_tensor_tensor(
                            out=t1[:], in0=xact[part][:], scalar=sd["dpc"][:, part:part + 1],
                            in1=ypsum[:], op0=ALU.mult, op1=ALU.add)
                        nc.vector.tensor_mul(z[part][:], t1[:], sgz[:])

                    # ---- POST: out_proj + residual/8 -> RS -> AG ----
                    for n0, nn in NTS:
                        for mt in range(KT):
                            po = pmm.tile([128, 512], F32, tag="mm", name="po")
                            nc.tensor.matmul(po[:, :nn],
                                             lhsT=sd["wout"][:, mt * 128:(mt + 1) * 128],
                                             rhs=z[0][:, n0:n0 + nn], start=True, stop=False)
                            nc.tensor.matmul(po[:, :nn],
                                             lhsT=sd["wout"][0:64, D + mt * 128:D + (mt + 1) * 128],
                                             rhs=z[1][0:64, n0:n0 + nn], start=False, stop=True)
                            res = pres.tile([128, 512], F16, tag="res", name="res")
                            nc.sync.dma_start(
                                res[:, :nn],
                                res_src[(dr, l)][mt * 128:(mt + 1) * 128, n0:n0 + nn])
                            so = pres.tile([128, 512], F16, tag="so", name="so")
                            nc.vector.scalar_tensor_tensor(
                                out=so[:, :nn], in0=res[:, :nn], scalar=0.125,
                                in1=po[:, :nn], op0=ALU.mult, op1=ALU.add)
                            nc.sync.dma_start(bnc[p + "hp_i"][mt * 128:(mt + 1) * 128, n0:n0 + nn],
                                              so[:, :nn])
                    nc.gpsimd.collective_compute(
                        "ReduceScatter", ALU.add, replica_groups=rg,
                        ins=[bnc[p + "hp_i"][:].opt()], outs=[bnc[p + "hp_rs"][:].opt()])
                    nc.gpsimd.collective_compute(
                        "AllGather", ALU.bypass, replica_groups=rg,
                        ins=[bnc[p + "hp_rs"][:].opt()], outs=[bnc[p + "hp_o"][:].opt()])
                    hp_o[(dr, l)] = bnc[p + "hp_o"]
                    res_src[(dr, l + 1)] = bnc[p + "hp_o"]
                    sctx.close()
                lctx.close()
            rctx.close()  # scan patterns no longer needed

            # ------------- final: norms, replicated lm_head, logits -------------
            fctx = contextlib.ExitStack()
            pfin = fctx.enter_context(tc.tile_pool(name="pfin", bufs=KT))
            ppj = fctx.enter_context(tc.tile_pool(name="ppj", bufs=KT + 2))
            plm = fctx.enter_context(tc.tile_pool(name="plm", bufs=2))
            pjf, pjb = [], []
            for dr in ("f", "b"):
                lw = plm.tile([128, KT * D], F16, tag="lm", name=f"lm{dr}")
                nc.sync.dma_start(lw[:], P[f"lm_{dr}"][:])
                f1ctx = contextlib.ExitStack()
                pstr2 = f1ctx.enter_context(tc.tile_pool(name=f"pstr2{dr}", bufs=KT))
                ptmp2 = f1ctx.enter_context(tc.tile_pool(name=f"ptmp2{dr}", bufs=2))
                ptmp2b = f1ctx.enter_context(tc.tile_pool(name=f"ptmp2b{dr}", bufs=1))
                hAR = hp_o[(dr, NL - 1)]
                hsb = [pstr2.tile([128, L], F16, tag="hs2", name="hs2") for _ in range(KT)]
                for kt in range(KT):
                    nc.sync.dma_start(hsb[kt][:], hAR[kt * 128:(kt + 1) * 128, :])
                invr = rmsnorm_invr(hsb, ptmp2, ptmp2b)
                hnf = [pfin.tile([128, L], F16, tag="hnf", name="hnf") for _ in range(KT)]
                for kt in range(KT):
                    if dr == "f":
                        nc.vector.tensor_mul(hnf[kt][:], hsb[kt][:], invr[:])
                    else:  # un-flip along t
                        nc.vector.tensor_mul(hnf[kt][:], hsb[kt][:, ::-1], invr[:, ::-1])
                # projection: proj_f stored; proj_b added on top
                for mt in range(KT):
                    pj = ppj.tile([128, L], F16, tag="pj", name="pj")
                    (pjf if dr == "f" else pjb).append(pj)
                    for n0, nn in NTS:
                        pp = pmm.tile([128, 512], F32, tag="mm", name="pp")
                        for kt in range(KT):
                            nc.tensor.matmul(pp[:, :nn],
                                             lhsT=lw[:, kt * D + mt * 128: kt * D + (mt + 1) * 128],
                                             rhs=hnf[kt][:, n0:n0 + nn],
                                             start=(kt == 0), stop=(kt == KT - 1))
                        if dr == "f":
                            nc.scalar.activation(pj[:, n0:n0 + nn], pp[:, :nn], AF.Copy)
                        else:
                            nc.vector.tensor_tensor(out=pj[:, n0:n0 + nn],
                                                    in0=pjf[mt][:, n0:n0 + nn],
                                                    in1=pp[:, :nn], op=ALU.add)
                f1ctx.close()

            # logits: embT (vocab shard) @ pjb, weights streamed per M-tile
            pembL = fctx.enter_context(tc.tile_pool(name="pembL", bufs=3))
            for mt in range(MV):
                embTm = pembL.tile([128, KT * 128], F16, tag="embTm", name="embTm")
                nc.sync.dma_start(embTm[:], P["embT"][:, mt * KT * 128:(mt + 1) * KT * 128])
                for n0, nn in NTS:
                    pl = pmm.tile([128, 512], F32, tag="mm", name="pl")
                    for kt in range(KT):
                        nc.tensor.matmul(
                            pl[:, :nn],
                            lhsT=embTm[:, kt * 128:(kt + 1) * 128],
                            rhs=pjb[kt][:, n0:n0 + nn],
                            start=(kt == 0), stop=(kt == KT - 1))
                    sl = pres.tile([128, 512], F32, tag="sl", name="sl")
                    if mt % 2 == 0:
                        nc.scalar.activation(sl[:, :nn], pl[:, :nn], AF.Copy)
                    else:
                        nc.vector.tensor_copy(sl[:, :nn], pl[:, :nn])
                    nc.sync.dma_start(out_ext[mt * 128:(mt + 1) * 128, n0:n0 + nn],
                                      sl[:, :nn])
            fctx.close()
    _split_waits(nc)
    return nc


# ====================== host side ======================

def _img_lhsT(w):
    """(Kdim, M) weight -> SBUF image (128, nkt*M) with K tiled by 128."""
    Kd, M = w.shape
    nkt = (Kd + 127) // 128
    img = np.zeros((128, nkt * M), np.float32)
    for kt in range(nkt):
        rows = min(128, Kd - kt * 128)
        img[:rows, kt * M:(kt + 1) * M] = w[kt * 128:kt * 128 + rows]
    return img


def _img_cols2(v):
    img = np.zeros((128, 2), np.float32)
    img[:, 0] = v[0:128]
    img[0:64, 1] = v[128:192]
    return img


def _prep_core(inputs, k, d):
    L, V, VP, D = d["L"], d["V"], d["VP"], d["D"]
    KT = D // 128
    LCH = L // 128
    ids = np.asarray(inputs["input_ids"]).reshape(L).astype(np.int32)
    emb = np.asarray(inputs["embedding"], np.float32)
    m = {}
    m["ids"] = np.ascontiguousarray(ids.reshape(LCH, 128).T)
    m["emb"] = emb
    embP = np.zeros((VP, D), np.float32)
    embP[:V] = emb
    shard = embP[k * (VP // NC):(k + 1) * (VP // NC)]   # (MVW, D)
    MVL = shard.shape[0] // 128
    img = np.zeros((128, MVL * KT * 128), np.float32)
    for mt_ in range(MVL):
        for kt_ in range(KT):
            blk = shard[mt_ * 128:(mt_ + 1) * 128, kt_ * 128:(kt_ + 1) * 128].T
            img[:, (mt_ * KT + kt_) * 128:(mt_ * KT + kt_ + 1) * 128] = blk
    m["embT"] = img

    c0, c1 = k * DSH, (k + 1) * DSH
    for dr in ("f", "b"):
        for l in range(NL):
            p = f"{dr}{l}_"
            g = lambda nm: np.asarray(inputs[f"{dr}_{nm}"][l], np.float32)
            W = np.concatenate([g("in_proj")[c0:c1], g("in_proj")[DI + c0:DI + c1]], 0)
            W = W * np.asarray(inputs[f"{dr}_norm_w"][l], np.float32)[None, :]
            m[p + "win"] = _img_lhsT(np.ascontiguousarray(W.T))
            m[p + "wout"] = _img_lhsT(np.ascontiguousarray(g("out_proj")[:, c0:c1].T))
            xpT = np.ascontiguousarray(g("x_proj")[:, c0:c1].T)   # (192, 80)
            xpP = np.zeros((DSH, DBCR), np.float32)
            xpP[:, 0:S] = xpT[:, R:R + S]
            xpP[:, 32:32 + S] = xpT[:, R + S:R + 2 * S]
            xpP[:, 64:64 + R] = xpT[:, 0:R]
            m[p + "wx"] = _img_lhsT(xpP)
            wdtP = np.zeros((DBCR, DSH), np.float32)
            wdtP[64:64 + R] = g("dt_w")[c0:c1].T
            m[p + "wdt"] = wdtP
            m[p + "dtb"] = _img_cols2(g("dt_b")[c0:c1])
            cwk = g("conv_w")[c0:c1]
            m[p + "cw"] = np.zeros((128, 2 * K), np.float32)
            m[p + "cw"][:, 0:K] = cwk[0:128]
            m[p + "cw"][0:64, K:2 * K] = cwk[128:192]
            m[p + "cb"] = _img_cols2(g("conv_b")[c0:c1])
            m[p + "dpc"] = _img_cols2(g("Dp")[c0:c1])
    lm = np.asarray(inputs["lm_head_proj"], np.float32)
    nf_f = np.asarray(inputs["f_norm_f"], np.float32)
    nf_b = np.asarray(inputs["b_norm_f"], np.float32)
    m["lm_f"] = _img_lhsT(np.ascontiguousarray((lm[:, :D] * nf_f[None, :]).T))
    m["lm_b"] = _img_lhsT(np.ascontiguousarray((lm[:, D:] * nf_b[None, :]).T))

    # patterns: scan-tile row m -> (dloc = m//16, s = m%16); channel-group j
    pat_dA = np.zeros((128, NJ * 128), np.float32)
    pat_rep = np.zeros((128, NJ * 128), np.float32)
    pat_sum = np.zeros((128, NJ * 128), np.float32)
    pat_B = np.zeros((48, 128), np.float32)
    for mm_ in range(128):
        dloc, s = mm_ // 16, mm_ % 16
        pat_B[s, mm_] = 1.0
        pat_B[32 + s, mm_] = 1.0
        for j in range(NJ):
            krow = (8 * j + dloc) % 128     # row of delta/du half tile
            pat_dA[krow, j * 128 + mm_] = -(s + 1)
            pat_rep[krow, j * 128 + mm_] = 1.0
    for r in range(128):
        dloc = r // 16
        for j in range(NJ):
            mrow = (8 * j + dloc) % 128     # row of ypsum
            pat_sum[r, j * 128 + mrow] = 1.0
    m["pat_dA"], m["pat_rep"], m["pat_sum"], m["pat_B"] = pat_dA, pat_rep, pat_sum, pat_B
    f16keys = ["embT", "lm_f", "lm_b", "pat_dA", "pat_rep", "pat_sum", "pat_B"]
    for dr in ("f", "b"):
        for l in range(NL):
            pp_ = f"{dr}{l}_"
            f16keys += [pp_ + "win", pp_ + "wout", pp_ + "wx", pp_ + "wdt"]
    for k_ in f16keys:
        m[k_] = m[k_].astype(np.float16)
    return m


_NC_CACHE = {}
TRACE = False
LAST_EXEC_NS = None
LAST_RESULTS = None


def kernel(**inputs):
    global LAST_EXEC_NS, LAST_RESULTS
    d = dims()
    key = "small" if SMALL else "full"
    if key not in _NC_CACHE:
        _NC_CACHE[key] = build_nc()
    ncg = _NC_CACHE[key]
    in_maps = [_prep_core(inputs, k, d) for k in range(NC)]
    res = run_bass_kernel_spmd(ncg, in_maps, core_ids=list(range(NC)), trace=TRACE)
    LAST_EXEC_NS = res.exec_time_ns
    LAST_RESULTS = res
    L, V, VP = d["L"], d["V"], d["VP"]
    full = np.concatenate([res.results[k]["out"] for k in range(NC)], axis=0)  # (VP, L)
    return np.ascontiguousarray(full[:V].T[None])


def timed_run(inputs, iters=4):
    """Measure per-call wall time of the compiled SPMD executable with
    pre-staged device inputs (no donation, no re-transfer). Returns
    (best_seconds, results_list)."""
    import time
    import jax
    from jax.sharding import Mesh, PartitionSpec
    from jax.experimental.shard_map import shard_map
    from concourse import bass2jax, mybir as mb

    d = dims()
    key = "small" if SMALL else "full"
    if key not in _NC_CACHE:
        _NC_CACHE[key] = build_nc()
    ncg = _NC_CACHE[key]
    in_maps = [_prep_core(inputs, k, d) for k in range(NC)]
    bass2jax.install_neuronx_cc_hook()
    partition_name = ncg.partition_id_tensor.name if ncg.partition_id_tensor else None
    in_names, out_names, out_avals, zero_outs = [], [], [], []
    for alloc in ncg.m.functions[0].allocations:
        if not isinstance(alloc, mb.MemoryLocationSet):
            continue
        name = alloc.memorylocations[0].name
        if alloc.kind == "ExternalInput":
            if name != partition_name:
                in_names.append(name)
        elif alloc.kind == "ExternalOutput":
            shape = tuple(alloc.tensor_shape)
            dtype = mb.dt.np(alloc.dtype)
            out_names.append(name)
            out_avals.append(jax.core.ShapedArray(shape, dtype))
            zero_outs.append(np.zeros(shape, dtype))
    n_params = len(in_names)
    all_names = in_names + out_names
    if partition_name is not None:
        all_names = all_names + [partition_name]

    def _body(*args):
        operands = list(args)
        if partition_name is not None:
            operands.append(bass2jax.partition_id_tensor())
        outs = bass2jax._bass_exec_p.bind(
            *operands, out_avals=tuple(out_avals), in_names=tuple(all_names),
            out_names=tuple(out_names), lowering_input_output_aliases=(),
            sim_require_finite=True, sim_require_nnan=True, nc=ncg)
        return tuple(outs)

    devices = jax.devices()[:NC]
    mesh = Mesh(np.asarray(devices), ("core",))
    nin = n_params + len(zero_outs)
    sharded = jax.jit(shard_map(_body, mesh=mesh,
                                in_specs=(PartitionSpec("core"),) * nin,
                                out_specs=(PartitionSpec("core"),) * len(out_names),
                                check_rep=False), keep_unused=True)
    per_core = [[np.asarray(m[nm]) for nm in in_names] for m in in_maps]
    concat_in = [np.concatenate([per_core[c][i] for c in range(NC)], axis=0)
                 for i in range(n_params)]
    concat_zeros = [np.zeros((NC * z.shape[0], *z.shape[1:]), z.dtype)
                    for z in zero_outs]
    shardings = [jax.sharding.NamedSharding(mesh, PartitionSpec("core"))] * nin
    staged = [jax.device_put(a, s) for a, s in zip(concat_in + concat_zeros, shardings)]
    out = sharded(*staged)
    jax.block_until_ready(out)
    best = float("inf")
    for _ in range(iters):
        t0 = time.perf_counter()
        out = sharded(*staged)
        jax.block_until_ready(out)
        best = min(best, time.perf_counter() - t0)
    res = [{nm: np.asarray(out[i]).reshape(NC, *out_avals[i].shape)[c]
            for i, nm in enumerate(out_names)} for c in range(NC)]
    return best, res
